# revision 1
# baseline (speedup 1.0000x reference)
"""Trainium2 Bass kernel for CommunityPassing (segment mean + gather).

Algorithm (8 NeuronCores, data-parallel over nodes):
  host: shard x/community over 8 cores along the node axis; within each
        shard, stably sort node indices by community id and pack them into
        128-row tiles grouped by community "chunk" (128 communities per
        chunk, 8 chunks for 1000 communities). Pad each (core, chunk)
        block to a shared tile count so all cores run one SPMD program.
  dev:  phase 1 - stream sorted x tiles; build a per-tile one-hot
        selection matrix B[node, local_comm] with a DVE is_equal against
        an iota row; matmul B^T @ x_tile accumulating into a PSUM tile
        per community chunk -> per-core partial community sums.
        AllReduce the [1024, 256] partial sums across the 8 cores,
        multiply by host-computed 1/count, write the [1024, 256]
        community-mean table to DRAM.
        phase 2 - dma_gather rows of the table with the original-order
        community ids (int16) and stream the result to the output.
  host: concatenate the 8 output shards.
"""

import os
import sys

import numpy as np

for _p in ("/opt/trn_rl_repo", "/opt/pypackages"):
    if _p not in sys.path and os.path.isdir(_p):
        sys.path.append(_p)

# Problem constants (hardcoded per the task contract).
N_FULL = 500000
F = 256
NUM_COMMS = 1000
EPS = 1e-12
M = 8               # cores
P = 128             # partitions
NC_CHUNKS = 8       # community chunks of 128 (8*128 = 1024 >= 1000)
GATHER_BATCH = 2048  # rows per dma_gather (multiple of 128)
XB = 8              # x tiles per streaming DMA (8 * 128KB = 1MB)
JB = GATHER_BATCH // P

# Stash of the most recent run's BassKernelResults (for test harnesses).
LAST_RESULTS = None


def _host_prep(x, community):
    """Build per-core device inputs. Returns (in_maps, plan)."""
    x = np.ascontiguousarray(np.asarray(x, dtype=np.float32))
    community = np.asarray(community).astype(np.int64)
    n = x.shape[0]
    assert n % M == 0
    nl = n // M

    comm_sh = community.reshape(M, nl)
    perms = np.argsort(comm_sh, axis=1, kind="stable")
    comm_sorted = np.take_along_axis(comm_sh, perms, axis=1)

    # per (core, chunk) node counts
    chunk_ids = comm_sorted >> 7  # // 128
    cnts = np.zeros((M, NC_CHUNKS), dtype=np.int64)
    for m in range(M):
        bc = np.bincount(chunk_ids[m], minlength=NC_CHUNKS)
        cnts[m] = bc[:NC_CHUNKS]
    t_k = np.maximum(1, -(-cnts.max(axis=0) // P))  # ceil, shared by all cores
    t_total = int(t_k.sum())
    chunk_of_tile = np.repeat(np.arange(NC_CHUNKS), t_k)
    tile_off = np.concatenate([[0], np.cumsum(t_k)])  # tile index base per chunk

    # counts -> 1/max(cnt, eps), [p, k] layout (community id = k*128 + p)
    cnt_full = np.bincount(community, minlength=NUM_COMMS).astype(np.float32)
    inv_pad = np.zeros((NC_CHUNKS * P,), np.float32)
    inv_pad[:NUM_COMMS] = 1.0 / np.maximum(cnt_full, np.float32(EPS))
    invc = np.ascontiguousarray(inv_pad.reshape(NC_CHUNKS, P).T)  # [128, 8]

    iota = np.ascontiguousarray(
        np.tile(np.arange(P, dtype=np.float32), (P, 1))
    )  # [128, 128], each row 0..127
    import ml_dtypes

    ident = np.eye(P).astype(ml_dtypes.bfloat16)

    in_maps = []
    origs = []
    for m in range(M):
        x_m = x[m * nl : (m + 1) * nl]
        xs = np.zeros((t_total * P, F), dtype=np.float32)
        locid = np.full((t_total * P,), -1.0, dtype=np.float32)
        orig = np.full((t_total * P,), -1, dtype=np.int64)
        start = 0
        for k in range(NC_CHUNKS):
            c = int(cnts[m, k])
            row = int(tile_off[k]) * P
            sel = perms[m, start : start + c]
            xs[row : row + c] = x_m[sel]
            orig[row : row + c] = sel
            locid[row : row + c] = comm_sorted[m, start : start + c] - k * P
            start += c
        locid_t = np.ascontiguousarray(locid.reshape(t_total, P).T)  # [128, T]
        origs.append(orig)

        xs_hi = xs.astype(ml_dtypes.bfloat16)
        xs_lo = (xs - xs_hi.astype(np.float32)).astype(ml_dtypes.bfloat16)
        in_maps.append(
            {
                "xs_hi": xs_hi,
                "xs_lo": xs_lo,
                "locid": locid_t,
                "iota": iota,
                "ident": ident,
                "invc": invc,
            }
        )

    plan = {
        "nl": nl,
        "t_k": [int(v) for v in t_k],
        "t_total": t_total,
        "chunk_of_tile": [int(v) for v in chunk_of_tile],
        "tile_off": [int(v) for v in tile_off],
        "origs": origs,
    }
    return in_maps, plan


def _build_program(plan, use_collective=True, use_gather=True):
    from concourse import bacc, mybir, tile

    t_total = plan["t_total"]
    chunk_of_tile = plan["chunk_of_tile"]
    tile_off = plan["tile_off"]

    dt = mybir.dt
    nc = bacc.Bacc("TRN2", target_bir_lowering=False, debug=False, num_devices=M)

    xs_hi = nc.dram_tensor("xs_hi", [t_total * P, F], dt.bfloat16, kind="ExternalInput")
    xs_lo = nc.dram_tensor("xs_lo", [t_total * P, F], dt.bfloat16, kind="ExternalInput")
    locid = nc.dram_tensor("locid", [P, t_total], dt.float32, kind="ExternalInput")
    iota = nc.dram_tensor("iota", [P, P], dt.float32, kind="ExternalInput")
    ident = nc.dram_tensor("ident", [P, P], dt.bfloat16, kind="ExternalInput")
    invc = nc.dram_tensor("invc", [P, NC_CHUNKS], dt.float32, kind="ExternalInput")
    out = nc.dram_tensor("out", [t_total * P, F], dt.float32, kind="ExternalOutput")

    xs_hi_view = xs_hi.ap().rearrange("(t p) f -> p t f", p=P)  # [128, T, 256]
    xs_lo_view = xs_lo.ap().rearrange("(t p) f -> p t f", p=P)

    with tile.TileContext(nc) as tc:
        with (
            tc.tile_pool(name="const", bufs=1) as constp,
            tc.tile_pool(name="xsp", bufs=3) as xsp,
            tc.tile_pool(name="bp", bufs=6) as bp,
            tc.tile_pool(name="acc", bufs=1) as accp,
            tc.tile_pool(name="psum", bufs=2, space="PSUM") as psp,
            tc.tile_pool(name="dram", bufs=1, space="DRAM") as dramp,
        ):
            iota_t = constp.tile([P, P], dt.float32)
            nc.sync.dma_start(out=iota_t[:], in_=iota.ap())
            ident_t = constp.tile([P, P], dt.bfloat16)
            nc.sync.dma_start(out=ident_t[:], in_=ident.ap())
            locid_t = constp.tile([P, t_total], dt.float32)
            nc.sync.dma_start(out=locid_t[:], in_=locid.ap())
            invc_t = constp.tile([P, NC_CHUNKS], dt.float32)
            nc.sync.dma_start(out=invc_t[:], in_=invc.ap())

            comm_sum = accp.tile([P, NC_CHUNKS * F], dt.float32)

            # ---- phase 1: streamed one-hot matmul segment sums ----
            xsb_hi = None
            xsb_lo = None
            bstart = 0
            psum_t = None
            for t in range(t_total):
                if t % XB == 0:
                    bstart = t
                    w = min(XB, t_total - t)
                    xsb_hi = xsp.tile([P, XB * F], dt.bfloat16, tag="xsbh")
                    nc.sync.dma_start(
                        out=xsb_hi[:, : w * F].rearrange("p (b f) -> p b f", b=w),
                        in_=xs_hi_view[:, t : t + w, :],
                    )
                    xsb_lo = xsp.tile([P, XB * F], dt.bfloat16, tag="xsbl")
                    nc.sync.dma_start(
                        out=xsb_lo[:, : w * F].rearrange("p (b f) -> p b f", b=w),
                        in_=xs_lo_view[:, t : t + w, :],
                    )
                k = chunk_of_tile[t]
                first = t == tile_off[k]
                last = t == tile_off[k + 1] - 1
                if first:
                    psum_t = psp.tile([P, F], dt.float32)
                b_t = bp.tile([P, P], dt.bfloat16, tag="b")
                nc.vector.tensor_scalar(
                    b_t[:],
                    iota_t[:],
                    locid_t[:, t : t + 1],
                    None,
                    mybir.AluOpType.is_equal,
                )
                j = t - bstart
                nc.tensor.matmul(
                    psum_t[:],
                    lhsT=b_t[:],
                    rhs=xsb_hi[:, j * F : (j + 1) * F],
                    start=first,
                    stop=False,
                )
                nc.tensor.matmul(
                    psum_t[:],
                    lhsT=b_t[:],
                    rhs=xsb_lo[:, j * F : (j + 1) * F],
                    start=False,
                    stop=last,
                )
                if last:
                    nc.vector.tensor_copy(
                        out=comm_sum[:, k * F : (k + 1) * F], in_=psum_t[:]
                    )

            # ---- all-reduce partial sums, scale by 1/count, write table ----
            ar_in = dramp.tile([P, NC_CHUNKS * F], dt.float32)
            ar_out = dramp.tile([P, NC_CHUNKS * F], dt.float32)
            nc.sync.dma_start(out=ar_in, in_=comm_sum[:])
            if use_collective:
                nc.gpsimd.collective_compute(
                    "AllReduce",
                    mybir.AluOpType.add,
                    replica_groups=[list(range(M))],
                    ins=[ar_in.opt()],
                    outs=[ar_out.opt()],
                )
            else:
                nc.sync.dma_start(out=ar_out, in_=ar_in)
            mean_sb = accp.tile([P, NC_CHUNKS * F], dt.float32)
            nc.sync.dma_start(out=mean_sb[:], in_=ar_out)
            for k in range(NC_CHUNKS):
                nc.vector.tensor_scalar(
                    mean_sb[:, k * F : (k + 1) * F],
                    mean_sb[:, k * F : (k + 1) * F],
                    invc_t[:, k : k + 1],
                    None,
                    mybir.AluOpType.mult,
                )

            # ---- phase 2: broadcast means back to (sorted) nodes ----
            # out_tile[node, f] = B[node, comm] @ mean_chunk[comm, f];
            # matmul wants lhsT = B^T, produced by a PE transpose.
            # fp32 matmul streams at 1/4 rate, so split the mean into two
            # bf16 limbs (hi + residual) and run two full-rate bf16 matmuls
            # accumulating in fp32 PSUM (~16-bit-exact result).
            mean_hi = accp.tile([P, NC_CHUNKS * F], dt.bfloat16)
            mean_lo = accp.tile([P, NC_CHUNKS * F], dt.bfloat16)
            mean_rest = accp.tile([P, NC_CHUNKS * F], dt.float32)
            nc.vector.tensor_copy(out=mean_hi[:], in_=mean_sb[:])
            nc.vector.tensor_copy(out=mean_rest[:], in_=mean_hi[:])
            nc.vector.tensor_tensor(
                out=mean_rest[:],
                in0=mean_sb[:],
                in1=mean_rest[:],
                op=mybir.AluOpType.subtract,
            )
            nc.vector.tensor_copy(out=mean_lo[:], in_=mean_rest[:])
            out_view = out.ap().rearrange("(t p) f -> p t f", p=P)
            with (
                tc.tile_pool(name="pst", bufs=2, space="PSUM") as pst,
                tc.tile_pool(name="pso", bufs=2, space="PSUM") as pso,
                tc.tile_pool(name="btp", bufs=4) as btp,
                tc.tile_pool(name="outp", bufs=3) as outp,
            ):
                if not use_gather:
                    t_total2 = 0
                else:
                    t_total2 = t_total
                outsb = None
                for t in range(t_total2):
                    if t % XB == 0:
                        outsb = outp.tile([P, XB * F], dt.float32, tag="outsb")
                    k = chunk_of_tile[t]
                    b2 = bp.tile([P, P], dt.bfloat16, tag="b2")
                    nc.vector.tensor_scalar(
                        b2[:],
                        iota_t[:],
                        locid_t[:, t : t + 1],
                        None,
                        mybir.AluOpType.is_equal,
                    )
                    bt_ps = pst.tile([P, P], dt.bfloat16)
                    nc.tensor.transpose(out=bt_ps[:], in_=b2[:], identity=ident_t[:])
                    bt_sb = btp.tile([P, P], dt.bfloat16, tag="bt")
                    nc.scalar.copy(out=bt_sb[:], in_=bt_ps[:])
                    op_ps = pso.tile([P, F], dt.float32)
                    nc.tensor.matmul(
                        op_ps[:],
                        lhsT=bt_sb[:],
                        rhs=mean_hi[:, k * F : (k + 1) * F],
                        start=True,
                        stop=False,
                    )
                    nc.tensor.matmul(
                        op_ps[:],
                        lhsT=bt_sb[:],
                        rhs=mean_lo[:, k * F : (k + 1) * F],
                        start=False,
                        stop=True,
                    )
                    j = t % XB
                    nc.vector.tensor_copy(
                        out=outsb[:, j * F : (j + 1) * F], in_=op_ps[:]
                    )
                    if t % XB == XB - 1 or t == t_total2 - 1:
                        t0 = t - j
                        w = j + 1
                        nc.sync.dma_start(
                            out=out_view[:, t0 : t0 + w, :],
                            in_=outsb[:, : w * F].rearrange(
                                "p (b f) -> p b f", b=w
                            ),
                        )

    nc.compile()
    return nc


def kernel(x, community):
    global LAST_RESULTS
    from concourse.bass_utils import run_bass_kernel_spmd

    in_maps, plan = _host_prep(x, community)
    nc = _build_program(plan)
    res = run_bass_kernel_spmd(nc, in_maps, core_ids=list(range(M)))
    LAST_RESULTS = res
    nl = plan["nl"]
    outs = []
    for m in range(M):
        out_sorted = res.results[m]["out"]
        orig = plan["origs"][m]
        valid = orig >= 0
        out_m = np.empty((nl, F), dtype=np.float32)
        out_m[orig[valid]] = out_sorted[valid]
        outs.append(out_m)
    return np.concatenate(outs, axis=0)



# revision 4
# speedup vs baseline: 1.6954x; 1.6954x over previous
"""Trainium2 Bass kernel for CommunityPassing (segment mean + gather).

Algorithm (8 NeuronCores, data-parallel over nodes):
  host: shard x/community over 8 cores along the node axis; within each
        shard, stably sort node indices by community id and pack them into
        128-row tiles grouped by community "chunk" (128 communities per
        chunk, 8 chunks for 1000 communities). Pad each (core, chunk)
        block to a shared tile count so all cores run one SPMD program.
        Cast x to bf16 and lay tiles out in 16-tile blocks with
        per-partition-contiguous 8KB rows for full-rate DMA.
  dev:  phase 1 - stream sorted x blocks; per tile build a one-hot
        B[node, local_comm] with a DVE is_equal; matmul B^T @ x_tile
        accumulating in PSUM per community chunk -> per-core partial sums
        (bf16).  AllReduce the sums in two 256KB halves (chunks 0-3 and
        4-7) so the collectives overlap with the phase-1 read stream.
        Scale by host-computed 1/count -> community-mean table (bf16).
        phase 2 - per 4-tile group, broadcast the group's local ids
        across partitions with a K=1 matmul (ones^T @ locid_row), build
        B^T directly with one is_equal against the partition index, then
        per tile matmul B^T^T... (lhsT=B^T) @ mean_chunk -> out tile.
        Copy PSUM->SBUF (bf16) on the Scalar engine and store 16-tile
        1MB blocks.  Phase-2 groups are interleaved into phase 1's tail
        so collective latency and engine work hide under the DMA stream.
  host: unpack blocks, scatter rows back to original order, cast to f32.
"""

import os
import sys

import numpy as np

for _p in ("/opt/trn_rl_repo", "/opt/pypackages"):
    if _p not in sys.path and os.path.isdir(_p):
        sys.path.append(_p)

# Problem constants (hardcoded per the task contract).
N_FULL = 500000
F = 256
NUM_COMMS = 1000
EPS = 1e-12
M = 8               # cores
P = 128             # partitions
NC_CHUNKS = 8       # community chunks of 128 (8*128 = 1024 >= 1000)
BT = 16             # tiles per x/out DMA block (8KB per partition = 1MB)
G = 4               # tiles per phase-2 B^T build group
LAG_TILES = 128     # phase-1 tiles between AR issue and first dependent p2 op

# Stash of the most recent run's BassKernelResults (for test harnesses).
LAST_RESULTS = None


def _host_prep(x, community):
    """Build per-core device inputs. Returns (in_maps, plan)."""
    import ml_dtypes

    bf16 = ml_dtypes.bfloat16
    x = np.ascontiguousarray(np.asarray(x, dtype=np.float32))
    community = np.asarray(community).astype(np.int64)
    n = x.shape[0]
    assert n % M == 0
    nl = n // M

    comm_sh = community.reshape(M, nl)
    perms = np.argsort(comm_sh, axis=1, kind="stable")
    comm_sorted = np.take_along_axis(comm_sh, perms, axis=1)

    # per (core, chunk) node counts
    chunk_ids = comm_sorted >> 7  # // 128
    cnts = np.zeros((M, NC_CHUNKS), dtype=np.int64)
    for m in range(M):
        bc = np.bincount(chunk_ids[m], minlength=NC_CHUNKS)
        cnts[m] = bc[:NC_CHUNKS]
    t_k = np.maximum(1, -(-cnts.max(axis=0) // P))  # ceil, shared by all cores
    # pad the total tile count to a multiple of BT (extend the last chunk)
    t_total = int(t_k.sum())
    pad = (-t_total) % BT
    t_k[-1] += pad
    t_total += pad
    chunk_of_tile = np.repeat(np.arange(NC_CHUNKS), t_k)
    tile_off = np.concatenate([[0], np.cumsum(t_k)])  # tile index base per chunk

    # counts -> 1/max(cnt, eps), [p, k] layout (community id = k*128 + p)
    cnt_full = np.bincount(community, minlength=NUM_COMMS).astype(np.float32)
    inv_pad = np.zeros((NC_CHUNKS * P,), np.float32)
    inv_pad[:NUM_COMMS] = 1.0 / np.maximum(cnt_full, np.float32(EPS))
    invc = np.ascontiguousarray(inv_pad.reshape(NC_CHUNKS, P).T)  # [128, 8]

    iota_t = np.ascontiguousarray(
        np.tile(np.arange(P, dtype=np.float32), (P, 1))
    ).astype(bf16)  # [128, 128], each row 0..127
    iota_col = np.arange(P, dtype=np.float32).reshape(P, 1)  # [128, 1]
    ones_t = np.ones((P, P), dtype=bf16)

    nblk = t_total // BT
    ngrp = t_total // G
    roww = -(-ngrp // 3) * (G * P)  # locid_row cols per partition-slot

    in_maps = []
    origs = []
    for m in range(M):
        x_m = x[m * nl : (m + 1) * nl]
        xs = np.zeros((t_total * P, F), dtype=bf16)
        locid = np.full((t_total * P,), -1.0, dtype=np.float32)
        orig = np.full((t_total * P,), -1, dtype=np.int64)
        start = 0
        for k in range(NC_CHUNKS):
            c = int(cnts[m, k])
            row = int(tile_off[k]) * P
            sel = perms[m, start : start + c]
            xs[row : row + c] = x_m[sel]
            orig[row : row + c] = sel
            locid[row : row + c] = comm_sorted[m, start : start + c] - k * P
            start += c
        origs.append(orig)

        # blocked x layout: [nblk, P, BT*F] with 8KB contiguous per partition
        xs_blk = np.ascontiguousarray(
            xs.reshape(nblk, BT, P, F).transpose(0, 2, 1, 3)
        ).reshape(nblk * P, BT * F)
        # per-tile local ids, [128, T] (partition = node-in-tile)
        locid_t = np.ascontiguousarray(locid.reshape(t_total, P).T)
        # groups of G tiles packed at partitions {0,32,64,96} for the
        # K=1 broadcast matmul (tile_position constraint)
        locid_row = np.zeros((P, roww), dtype=bf16)
        lr = locid.reshape(ngrp, G * P).astype(bf16)
        for q in range(ngrp):
            part = (q % 3) * 32
            c0 = (q // 3) * (G * P)
            locid_row[part, c0 : c0 + G * P] = lr[q]

        in_maps.append(
            {
                "xs": xs_blk,
                "locid": locid_t,
                "locid_row": locid_row,
                "iota": iota_t,
                "iota_col": iota_col,
                "ones": ones_t,
                "invc": invc,
            }
        )

    plan = {
        "nl": nl,
        "t_total": t_total,
        "chunk_of_tile": [int(v) for v in chunk_of_tile],
        "tile_off": [int(v) for v in tile_off],
        "origs": origs,
        "roww": roww,
    }
    return in_maps, plan


def _build_program(plan, use_collective=True):
    from concourse import bacc, mybir, tile

    t_total = plan["t_total"]
    chunk_of_tile = plan["chunk_of_tile"]
    tile_off = plan["tile_off"]
    roww = plan["roww"]
    nblk = t_total // BT
    ngrp = t_total // G
    HALF_W = 4 * F  # 4 chunks of the community table per AR half

    dt = mybir.dt
    nc = bacc.Bacc("TRN2", target_bir_lowering=False, debug=False, num_devices=M)

    xs = nc.dram_tensor("xs", [nblk * P, BT * F], dt.bfloat16, kind="ExternalInput")
    locid = nc.dram_tensor("locid", [P, t_total], dt.float32, kind="ExternalInput")
    locid_row = nc.dram_tensor(
        "locid_row", [P, roww], dt.bfloat16, kind="ExternalInput"
    )
    iota = nc.dram_tensor("iota", [P, P], dt.bfloat16, kind="ExternalInput")
    iota_col = nc.dram_tensor("iota_col", [P, 1], dt.float32, kind="ExternalInput")
    ones = nc.dram_tensor("ones", [P, P], dt.bfloat16, kind="ExternalInput")
    invc = nc.dram_tensor("invc", [P, NC_CHUNKS], dt.float32, kind="ExternalInput")
    out = nc.dram_tensor("out", [nblk * P, BT * F], dt.bfloat16, kind="ExternalOutput")

    xs_view = xs.ap().rearrange("(b p) w -> b p w", p=P)  # [nblk, 128, BT*F]
    out_view = out.ap().rearrange("(b p) w -> b p w", p=P)

    with tile.TileContext(nc) as tc:
        with (
            tc.tile_pool(name="const", bufs=1) as constp,
            tc.tile_pool(name="acc", bufs=1) as accp,
            tc.tile_pool(name="xsp", bufs=3) as xsp,
            tc.tile_pool(name="bp", bufs=4) as bp,
            tc.tile_pool(name="btp", bufs=3) as btp,
            tc.tile_pool(name="outp", bufs=3) as outp,
            tc.tile_pool(name="ps1", bufs=2, space="PSUM") as ps1,
            tc.tile_pool(name="psB", bufs=2, space="PSUM") as psB,
            tc.tile_pool(name="psO", bufs=2, space="PSUM") as psO,
            tc.tile_pool(name="dram", bufs=1, space="DRAM") as dramp,
        ):
            iota_t = constp.tile([P, P], dt.bfloat16)
            nc.sync.dma_start(out=iota_t[:], in_=iota.ap())
            iota_col_t = constp.tile([P, 1], dt.float32)
            nc.sync.dma_start(out=iota_col_t[:], in_=iota_col.ap())
            ones_t = constp.tile([P, P], dt.bfloat16)
            nc.sync.dma_start(out=ones_t[:], in_=ones.ap())
            locid_t = constp.tile([P, t_total], dt.float32)
            nc.sync.dma_start(out=locid_t[:], in_=locid.ap())
            locid_row_t = constp.tile([P, roww], dt.bfloat16)
            nc.sync.dma_start(out=locid_row_t[:], in_=locid_row.ap())
            invc_t = constp.tile([P, NC_CHUNKS], dt.float32)
            nc.sync.dma_start(out=invc_t[:], in_=invc.ap())

            # per-core partial community sums (bf16) and the mean table
            comm_sum = accp.tile([P, NC_CHUNKS * F], dt.bfloat16)
            mean_sb = accp.tile([P, NC_CHUNKS * F], dt.bfloat16)

            ar_bufs = []
            for h in range(2):
                ar_in = dramp.tile([P, HALF_W], dt.bfloat16, name=f"ar_in{h}")
                ar_out = dramp.tile([P, HALF_W], dt.bfloat16, name=f"ar_out{h}")
                ar_bufs.append((ar_in, ar_out))

            def emit_ar_half(h):
                ar_in, ar_out = ar_bufs[h]
                o = h * HALF_W
                nc.sync.dma_start(out=ar_in, in_=comm_sum[:, o : o + HALF_W])
                if use_collective:
                    nc.gpsimd.collective_compute(
                        "AllReduce",
                        mybir.AluOpType.add,
                        replica_groups=[list(range(M))],
                        ins=[ar_in.opt()],
                        outs=[ar_out.opt()],
                    )
                else:
                    nc.sync.dma_start(out=ar_out, in_=ar_in)
                nc.sync.dma_start(out=mean_sb[:, o : o + HALF_W], in_=ar_out)
                for k in range(4 * h, 4 * h + 4):
                    nc.vector.tensor_scalar(
                        mean_sb[:, k * F : (k + 1) * F],
                        mean_sb[:, k * F : (k + 1) * F],
                        invc_t[:, k : k + 1],
                        None,
                        mybir.AluOpType.mult,
                    )

            # ---- phase-2 group emission (4 tiles per group) ----
            outsb = [None]

            def emit_p2_group(q):
                t0 = q * G
                bpart = (q % 3) * 32
                c0 = (q // 3) * (G * P)
                bc = psB.tile([P, G * P], dt.float32, tag="bc")
                nc.tensor.matmul(
                    bc[:],
                    lhsT=ones_t[bpart : bpart + 1, :],
                    rhs=locid_row_t[bpart : bpart + 1, c0 : c0 + G * P],
                    start=True,
                    stop=True,
                )
                btq = btp.tile([P, G * P], dt.bfloat16, tag="bt")
                nc.vector.tensor_scalar(
                    btq[:],
                    bc[:],
                    iota_col_t[:, 0:1],
                    None,
                    mybir.AluOpType.is_equal,
                )
                po = None
                for j2 in range(G):
                    t = t0 + j2
                    k = chunk_of_tile[t]
                    jb = t % BT
                    if jb == 0:
                        outsb[0] = outp.tile(
                            [P, BT * F], dt.bfloat16, tag="osb", name="osb"
                        )
                    if j2 % 2 == 0:
                        po = psO.tile([P, 2 * F], dt.float32, tag="po")
                    nc.tensor.matmul(
                        po[:, (j2 % 2) * F : (j2 % 2 + 1) * F],
                        lhsT=btq[:, j2 * P : (j2 + 1) * P],
                        rhs=mean_sb[:, k * F : (k + 1) * F],
                        start=True,
                        stop=True,
                    )
                    if j2 % 2 == 1:
                        nc.scalar.copy(
                            out=outsb[0][:, (jb - 1) * F : (jb + 1) * F],
                            in_=po[:],
                        )
                    if jb == BT - 1:
                        b = t // BT
                        nc.sync.dma_start(
                            out=out_view[b], in_=outsb[0][:]
                        )

            # ---- main emission loop: phase 1 with interleaved phase 2 ----
            xsb = None
            ps = None
            p2_next = 0
            ar_emitted = [False, False]
            ar_tile = [tile_off[4] - 1, tile_off[8] - 1]

            def p2_allowed_tiles():
                if ar_emitted[1]:
                    return t_total
                if ar_emitted[0]:
                    return tile_off[4]
                return 0

            for t in range(t_total):
                if t % BT == 0:
                    b = t // BT
                    xsb = xsp.tile([P, BT * F], dt.bfloat16, tag="xsb")
                    nc.sync.dma_start(out=xsb[:], in_=xs_view[b])
                k = chunk_of_tile[t]
                first = t == tile_off[k]
                last = t == tile_off[k + 1] - 1
                if first:
                    ps = ps1.tile([P, F], dt.float32, tag="ps")
                b_t = bp.tile([P, P], dt.bfloat16, tag="b")
                nc.vector.tensor_scalar(
                    b_t[:],
                    iota_t[:],
                    locid_t[:, t : t + 1],
                    None,
                    mybir.AluOpType.is_equal,
                )
                j = t % BT
                nc.tensor.matmul(
                    ps[:],
                    lhsT=b_t[:],
                    rhs=xsb[:, j * F : (j + 1) * F],
                    start=first,
                    stop=last,
                )
                if last:
                    nc.vector.tensor_copy(
                        out=comm_sum[:, k * F : (k + 1) * F], in_=ps[:]
                    )
                    if k == 3:
                        emit_ar_half(0)
                        ar_emitted[0] = True
                    elif k == 7:
                        emit_ar_half(1)
                        ar_emitted[1] = True
                # interleave phase-2 groups at a bounded rate: one group
                # per G phase-1 tiles, LAG_TILES after the AR was issued
                if (
                    ar_emitted[0]
                    and t >= ar_tile[0] + LAG_TILES
                    and t % G == 0
                    and p2_next < ngrp
                    and (p2_next + 1) * G <= p2_allowed_tiles()
                ):
                    emit_p2_group(p2_next)
                    p2_next += 1

            # tail: remaining phase-2 groups
            while p2_next < ngrp:
                if (p2_next + 1) * G > p2_allowed_tiles():
                    break
                emit_p2_group(p2_next)
                p2_next += 1
            assert p2_next == ngrp, (p2_next, ngrp)

    nc.compile()
    return nc


def kernel(x, community):
    global LAST_RESULTS
    from concourse.bass_utils import run_bass_kernel_spmd

    in_maps, plan = _host_prep(x, community)
    nc = _build_program(plan)
    res = run_bass_kernel_spmd(nc, in_maps, core_ids=list(range(M)))
    LAST_RESULTS = res
    nl = plan["nl"]
    t_total = plan["t_total"]
    nblk = t_total // BT
    outs = []
    for m in range(M):
        out_blk = np.asarray(res.results[m]["out"])  # [nblk*P, BT*F] bf16
        out_sorted = (
            out_blk.reshape(nblk, P, BT, F)
            .transpose(0, 2, 1, 3)
            .reshape(t_total * P, F)
        )
        orig = plan["origs"][m]
        valid = orig >= 0
        out_m = np.empty((nl, F), dtype=np.float32)
        out_m[orig[valid]] = out_sorted[valid]
        outs.append(out_m)
    return np.concatenate(outs, axis=0)


# revision 5
# speedup vs baseline: 1.7376x; 1.0249x over previous
"""Trainium2 Bass kernel for CommunityPassing (segment mean + gather).

Algorithm (8 NeuronCores, data-parallel over nodes):
  host: shard x/community over 8 cores along the node axis; within each
        shard, stably sort node indices by community id and pack them into
        128-row tiles grouped by community "chunk" (128 communities per
        chunk, 8 chunks for 1000 communities). Pad each (core, chunk)
        block to a shared tile count so all cores run one SPMD program.
        Cast x to bf16 and lay tiles out in 16-tile blocks with
        per-partition-contiguous 8KB rows for full-rate DMA.
  dev:  phase 1 - stream sorted x blocks; per tile build a one-hot
        B[node, local_comm] with a DVE is_equal; matmul B^T @ x_tile
        accumulating in PSUM per community chunk -> per-core partial sums
        (bf16).  AllReduce the sums in two 256KB halves (chunks 0-3 and
        4-7) so the collectives overlap with the phase-1 read stream.
        Scale by host-computed 1/count -> community-mean table (bf16).
        phase 2 - per 4-tile group, broadcast the group's local ids
        across partitions with a K=1 matmul (ones^T @ locid_row), build
        B^T directly with one is_equal against the partition index, then
        per tile matmul B^T^T... (lhsT=B^T) @ mean_chunk -> out tile.
        Copy PSUM->SBUF (bf16) on the Scalar engine and store 16-tile
        1MB blocks.  Phase-2 groups are interleaved into phase 1's tail
        so collective latency and engine work hide under the DMA stream.
  host: unpack blocks, scatter rows back to original order, cast to f32.
"""

import os
import sys

import numpy as np

for _p in ("/opt/trn_rl_repo", "/opt/pypackages"):
    if _p not in sys.path and os.path.isdir(_p):
        sys.path.append(_p)

# Problem constants (hardcoded per the task contract).
N_FULL = 500000
F = 256
NUM_COMMS = 1000
EPS = 1e-12
M = 8               # cores
P = 128             # partitions
NC_CHUNKS = 8       # community chunks of 128 (8*128 = 1024 >= 1000)
BT = 16             # tiles per x/out DMA block (8KB per partition = 1MB)
G = 4               # tiles per phase-2 B^T build group
LAG_TILES = 64     # phase-1 tiles between AR issue and first dependent p2 op

# Stash of the most recent run's BassKernelResults (for test harnesses).
LAST_RESULTS = None


def _host_prep(x, community):
    """Build per-core device inputs. Returns (in_maps, plan)."""
    import ml_dtypes

    bf16 = ml_dtypes.bfloat16
    x = np.ascontiguousarray(np.asarray(x, dtype=np.float32))
    community = np.asarray(community).astype(np.int64)
    n = x.shape[0]
    assert n % M == 0
    nl = n // M

    comm_sh = community.reshape(M, nl)
    perms = np.argsort(comm_sh, axis=1, kind="stable")
    comm_sorted = np.take_along_axis(comm_sh, perms, axis=1)

    # per (core, chunk) node counts
    chunk_ids = comm_sorted >> 7  # // 128
    cnts = np.zeros((M, NC_CHUNKS), dtype=np.int64)
    for m in range(M):
        bc = np.bincount(chunk_ids[m], minlength=NC_CHUNKS)
        cnts[m] = bc[:NC_CHUNKS]
    t_k = np.maximum(1, -(-cnts.max(axis=0) // P))  # ceil, shared by all cores
    # pad the total tile count to a multiple of BT (extend the last chunk)
    t_total = int(t_k.sum())
    pad = (-t_total) % BT
    t_k[-1] += pad
    t_total += pad
    chunk_of_tile = np.repeat(np.arange(NC_CHUNKS), t_k)
    tile_off = np.concatenate([[0], np.cumsum(t_k)])  # tile index base per chunk

    # counts -> 1/max(cnt, eps), [p, k] layout (community id = k*128 + p)
    cnt_full = np.bincount(community, minlength=NUM_COMMS).astype(np.float32)
    inv_pad = np.zeros((NC_CHUNKS * P,), np.float32)
    inv_pad[:NUM_COMMS] = 1.0 / np.maximum(cnt_full, np.float32(EPS))
    invc = np.ascontiguousarray(inv_pad.reshape(NC_CHUNKS, P).T)  # [128, 8]

    iota_t = np.ascontiguousarray(
        np.tile(np.arange(P, dtype=np.float32), (P, 1))
    ).astype(bf16)  # [128, 128], each row 0..127
    iota_col = np.arange(P, dtype=np.float32).reshape(P, 1)  # [128, 1]
    ones_t = np.ones((P, P), dtype=bf16)
    iota4 = np.ascontiguousarray(np.tile(iota_t, (1, G)))  # [128, G*128]

    nblk = t_total // BT
    ngrp = t_total // G
    roww = -(-ngrp // 3) * (G * P)  # locid_row cols per partition-slot

    in_maps = []
    origs = []
    for m in range(M):
        x_m = x[m * nl : (m + 1) * nl]
        xs = np.zeros((t_total * P, F), dtype=bf16)
        locid = np.full((t_total * P,), -1.0, dtype=np.float32)
        orig = np.full((t_total * P,), -1, dtype=np.int64)
        start = 0
        for k in range(NC_CHUNKS):
            c = int(cnts[m, k])
            row = int(tile_off[k]) * P
            sel = perms[m, start : start + c]
            xs[row : row + c] = x_m[sel]
            orig[row : row + c] = sel
            locid[row : row + c] = comm_sorted[m, start : start + c] - k * P
            start += c
        origs.append(orig)

        # blocked x layout: [nblk, P, BT*F] with 8KB contiguous per partition
        xs_blk = np.ascontiguousarray(
            xs.reshape(nblk, BT, P, F).transpose(0, 2, 1, 3)
        ).reshape(nblk * P, BT * F)
        # per-tile local ids, [128, T] (partition = node-in-tile)
        locid_t = np.ascontiguousarray(locid.reshape(t_total, P).T)
        # groups of G tiles packed at partitions {0,32,64,96} for the
        # K=1 broadcast matmul (tile_position constraint)
        locid_row = np.zeros((P, roww), dtype=bf16)
        lr = locid.reshape(ngrp, G * P).astype(bf16)
        for q in range(ngrp):
            part = (q % 3) * 32
            c0 = (q // 3) * (G * P)
            locid_row[part, c0 : c0 + G * P] = lr[q]

        in_maps.append(
            {
                "xs": xs_blk,
                "locid": locid_t,
                "locid_row": locid_row,
                "iota": iota_t,
                "iota4": iota4,
                "iota_col": iota_col,
                "ones": ones_t,
                "invc": invc,
            }
        )

    plan = {
        "nl": nl,
        "t_total": t_total,
        "chunk_of_tile": [int(v) for v in chunk_of_tile],
        "tile_off": [int(v) for v in tile_off],
        "origs": origs,
        "roww": roww,
    }
    return in_maps, plan


def _build_program(plan, use_collective=True):
    from concourse import bacc, mybir, tile

    t_total = plan["t_total"]
    chunk_of_tile = plan["chunk_of_tile"]
    tile_off = plan["tile_off"]
    roww = plan["roww"]
    nblk = t_total // BT
    ngrp = t_total // G
    HALF_W = 4 * F  # 4 chunks of the community table per AR half

    dt = mybir.dt
    nc = bacc.Bacc("TRN2", target_bir_lowering=False, debug=False, num_devices=M)

    xs = nc.dram_tensor("xs", [nblk * P, BT * F], dt.bfloat16, kind="ExternalInput")
    locid = nc.dram_tensor("locid", [P, t_total], dt.float32, kind="ExternalInput")
    locid_row = nc.dram_tensor(
        "locid_row", [P, roww], dt.bfloat16, kind="ExternalInput"
    )
    iota = nc.dram_tensor("iota", [P, P], dt.bfloat16, kind="ExternalInput")
    iota4 = nc.dram_tensor("iota4", [P, G * P], dt.bfloat16, kind="ExternalInput")
    iota_col = nc.dram_tensor("iota_col", [P, 1], dt.float32, kind="ExternalInput")
    ones = nc.dram_tensor("ones", [P, P], dt.bfloat16, kind="ExternalInput")
    invc = nc.dram_tensor("invc", [P, NC_CHUNKS], dt.float32, kind="ExternalInput")
    out = nc.dram_tensor("out", [nblk * P, BT * F], dt.bfloat16, kind="ExternalOutput")

    xs_view = xs.ap().rearrange("(b p) w -> b p w", p=P)  # [nblk, 128, BT*F]
    out_view = out.ap().rearrange("(b p) w -> b p w", p=P)

    with tile.TileContext(nc) as tc:
        with (
            tc.tile_pool(name="const", bufs=1) as constp,
            tc.tile_pool(name="acc", bufs=1) as accp,
            tc.tile_pool(name="xsp", bufs=3) as xsp,
            tc.tile_pool(name="bp", bufs=4) as bp,
            tc.tile_pool(name="btp", bufs=3) as btp,
            tc.tile_pool(name="outp", bufs=3) as outp,
            tc.tile_pool(name="ps1", bufs=2, space="PSUM") as ps1,
            tc.tile_pool(name="psB", bufs=2, space="PSUM") as psB,
            tc.tile_pool(name="psO", bufs=2, space="PSUM") as psO,
            tc.tile_pool(name="dram", bufs=1, space="DRAM") as dramp,
        ):
            iota_t = constp.tile([P, P], dt.bfloat16)
            nc.sync.dma_start(out=iota_t[:], in_=iota.ap())
            iota4_t = constp.tile([P, G * P], dt.bfloat16)
            nc.sync.dma_start(out=iota4_t[:], in_=iota4.ap())
            iota_col_t = constp.tile([P, 1], dt.float32)
            nc.sync.dma_start(out=iota_col_t[:], in_=iota_col.ap())
            ones_t = constp.tile([P, P], dt.bfloat16)
            nc.sync.dma_start(out=ones_t[:], in_=ones.ap())
            locid_t = constp.tile([P, t_total], dt.float32)
            nc.sync.dma_start(out=locid_t[:], in_=locid.ap())
            locid_row_t = constp.tile([P, roww], dt.bfloat16)
            nc.sync.dma_start(out=locid_row_t[:], in_=locid_row.ap())
            invc_t = constp.tile([P, NC_CHUNKS], dt.float32)
            nc.sync.dma_start(out=invc_t[:], in_=invc.ap())

            # per-core partial community sums (bf16) and the mean table
            comm_sum = accp.tile([P, NC_CHUNKS * F], dt.bfloat16)
            mean_sb = accp.tile([P, NC_CHUNKS * F], dt.bfloat16)

            ar_bufs = []
            for h in range(NC_CHUNKS):
                ar_in = dramp.tile([P, F], dt.bfloat16, name=f"ar_in{h}")
                ar_out = dramp.tile([P, F], dt.bfloat16, name=f"ar_out{h}")
                ar_bufs.append((ar_in, ar_out))

            # warm-up collective: absorbs first-collective setup + core skew
            # while the phase-1 read stream runs
            warm_in = dramp.tile([P, 8], dt.bfloat16, name="warm_in")
            warm_out = dramp.tile([P, 8], dt.bfloat16, name="warm_out")
            nc.sync.dma_start(out=warm_in, in_=ones_t[:, 0:8])
            if use_collective:
                nc.gpsimd.collective_compute(
                    "AllReduce",
                    mybir.AluOpType.add,
                    replica_groups=[list(range(M))],
                    ins=[warm_in.opt()],
                    outs=[warm_out.opt()],
                )

            def emit_ar_chunk(k):
                ar_in, ar_out = ar_bufs[k]
                o = k * F
                nc.sync.dma_start(out=ar_in, in_=comm_sum[:, o : o + F])
                if use_collective:
                    nc.gpsimd.collective_compute(
                        "AllReduce",
                        mybir.AluOpType.add,
                        replica_groups=[list(range(M))],
                        ins=[ar_in.opt()],
                        outs=[ar_out.opt()],
                    )
                else:
                    nc.sync.dma_start(out=ar_out, in_=ar_in)
                nc.sync.dma_start(out=mean_sb[:, o : o + F], in_=ar_out)
                nc.vector.tensor_scalar(
                    mean_sb[:, o : o + F],
                    mean_sb[:, o : o + F],
                    invc_t[:, k : k + 1],
                    None,
                    mybir.AluOpType.mult,
                )

            # ---- phase-2 group emission (4 tiles per group) ----
            outsb = [None]

            def emit_p2_group(q):
                t0 = q * G
                bpart = (q % 3) * 32
                c0 = (q // 3) * (G * P)
                bc = psB.tile([P, G * P], dt.float32, tag="bc")
                nc.tensor.matmul(
                    bc[:],
                    lhsT=ones_t[bpart : bpart + 1, :],
                    rhs=locid_row_t[bpart : bpart + 1, c0 : c0 + G * P],
                    start=True,
                    stop=True,
                )
                btq = btp.tile([P, G * P], dt.bfloat16, tag="bt")
                nc.vector.tensor_scalar(
                    btq[:],
                    bc[:],
                    iota_col_t[:, 0:1],
                    None,
                    mybir.AluOpType.is_equal,
                )
                po = None
                for j2 in range(G):
                    t = t0 + j2
                    k = chunk_of_tile[t]
                    jb = t % BT
                    if jb == 0:
                        outsb[0] = outp.tile(
                            [P, BT * F], dt.bfloat16, tag="osb", name="osb"
                        )
                    if j2 % 2 == 0:
                        po = psO.tile([P, 2 * F], dt.float32, tag="po")
                    nc.tensor.matmul(
                        po[:, (j2 % 2) * F : (j2 % 2 + 1) * F],
                        lhsT=btq[:, j2 * P : (j2 + 1) * P],
                        rhs=mean_sb[:, k * F : (k + 1) * F],
                        start=True,
                        stop=True,
                    )
                    if j2 % 2 == 1:
                        nc.scalar.copy(
                            out=outsb[0][:, (jb - 1) * F : (jb + 1) * F],
                            in_=po[:],
                        )
                    if jb == BT - 1:
                        b = t // BT
                        nc.sync.dma_start(
                            out=out_view[b], in_=outsb[0][:]
                        )

            # ---- main emission loop: phase 1 with interleaved phase 2 ----
            xsb = None
            ps = None
            b4 = None
            p2_next = 0
            ar_emit_tile = {}

            for t in range(t_total):
                if t % BT == 0:
                    b = t // BT
                    xsb = xsp.tile([P, BT * F], dt.bfloat16, tag="xsb")
                    nc.sync.dma_start(out=xsb[:], in_=xs_view[b])
                if t % G == 0:
                    # batched one-hot build for G phase-1 tiles in one DVE op
                    b4 = bp.tile([P, G * P], dt.bfloat16, tag="b")
                    nc.vector.tensor_tensor(
                        out=b4[:].rearrange("p (g j) -> p g j", g=G),
                        in0=iota4_t[:].rearrange("p (g j) -> p g j", g=G),
                        in1=locid_t[:, t : t + G]
                        .unsqueeze(2)
                        .broadcast_to([P, G, P]),
                        op=mybir.AluOpType.is_equal,
                    )
                k = chunk_of_tile[t]
                first = t == tile_off[k]
                last = t == tile_off[k + 1] - 1
                if first:
                    ps = ps1.tile([P, F], dt.float32, tag="ps")
                j = t % BT
                jg = t % G
                nc.tensor.matmul(
                    ps[:],
                    lhsT=b4[:, jg * P : (jg + 1) * P],
                    rhs=xsb[:, j * F : (j + 1) * F],
                    start=first,
                    stop=last,
                )
                if last:
                    nc.vector.tensor_copy(
                        out=comm_sum[:, k * F : (k + 1) * F], in_=ps[:]
                    )
                    emit_ar_chunk(k)
                    ar_emit_tile[k] = t
                # interleave phase-2 groups at a bounded rate: one group per
                # G phase-1 tiles, LAG_TILES after that chunk's AR was issued
                if t % G == G - 1 and p2_next < ngrp:
                    klast = chunk_of_tile[p2_next * G + G - 1]
                    if (
                        klast in ar_emit_tile
                        and t >= ar_emit_tile[klast] + LAG_TILES
                    ):
                        emit_p2_group(p2_next)
                        p2_next += 1

            # tail: remaining phase-2 groups
            while p2_next < ngrp:
                emit_p2_group(p2_next)
                p2_next += 1

    nc.compile()
    return nc


def kernel(x, community):
    global LAST_RESULTS
    from concourse.bass_utils import run_bass_kernel_spmd

    in_maps, plan = _host_prep(x, community)
    nc = _build_program(plan)
    res = run_bass_kernel_spmd(nc, in_maps, core_ids=list(range(M)))
    LAST_RESULTS = res
    nl = plan["nl"]
    t_total = plan["t_total"]
    nblk = t_total // BT
    outs = []
    for m in range(M):
        out_blk = np.asarray(res.results[m]["out"])  # [nblk*P, BT*F] bf16
        out_sorted = (
            out_blk.reshape(nblk, P, BT, F)
            .transpose(0, 2, 1, 3)
            .reshape(t_total * P, F)
        )
        orig = plan["origs"][m]
        valid = orig >= 0
        out_m = np.empty((nl, F), dtype=np.float32)
        out_m[orig[valid]] = out_sorted[valid]
        outs.append(out_m)
    return np.concatenate(outs, axis=0)
